# revision 2
# baseline (speedup 1.0000x reference)
"""MoE FFN (8 routed experts top-2 + 1 shared) on 8 TRN2 NeuronCores.

Expert-parallel token dispatch with split-fp8 DoubleRow matmuls (hi+lo
e4m3 pairs, 3 DR matmuls per 256-contraction: 4/3 over bf16 at bf16
accuracy). Router/combine weights on host. v3 over v2:
 - a2a halves rebalanced (JH=88: 704/448 rows) so the second, critical
   collective is smaller and starts as early as possible
 - shared-expert weight streams are chunked and interleaved with the
   second down phase so a2a_in stores don't queue behind them
 - startup loads trimmed (first up/gate weight chunk is 128 cols)
 - combine via dma_gather of the two expert rows per token + DVE adds
   (removes the one-hot combine matmul from the PE critical path)
"""

import ml_dtypes
import numpy as np

import concourse.bacc as bacc
import concourse.bass as bass
import concourse.mybir as mybir
import concourse.tile as tile
from concourse.bass_utils import run_bass_kernel_spmd

P = 128
C = 1024            # d_model
H = 2048            # d_expert
T = 4096            # tokens
E = 8               # routed experts == cores
TS = T // E         # 512 tokens per core
CC = C // P         # 8 k-blocks for up/gate contraction
CPAIR = CC // 2     # 4 DoubleRow pairs
HC = H // P         # 16 k-blocks for down contraction
HPAIR = HC // 2     # 8 DoubleRow pairs
LC = 138            # per (expert, owner) capacity (host balances ownership)
S = E * LC          # 1104 slots per expert core
SCH = (S + P - 1) // P      # 9 slot chunks (last partial: 80 slots)
SP_ = SCH * P       # 1152: padded slot count for [P, SCH] host layouts
JSPLIT = (48, 48, 42)       # per-owner slots per a2a chunk (sum = LC)
RSTART = (0, 384, 768, 1104)    # chunk row boundaries (E * cumsum)
PHASE_SC = ((0, 3), (3, 6), (6, 9))   # down slot-chunks per phase
PHASE_UG = ((0, 384), (384, 768), (768, 1104))

NCORES = 8
F32 = mybir.dt.float32
BF16 = mybir.dt.bfloat16
I16 = mybir.dt.int16
F8 = mybir.dt.float8e4
DR = mybir.MatmulPerfMode.DoubleRow
ACTF = mybir.ActivationFunctionType
OP = mybir.AluOpType
E4NP = ml_dtypes.float8_e4m3

SU = 64.0           # Wup scale
SG = 8.0            # Wgate scale (act tiles carry this scale)
SD = 64.0           # Wdown scale
SY = SG * SD        # 512: PSUM y scale


def _build_program():
    nc = bacc.Bacc("TRN2", target_bir_lowering=False, debug=False,
                   num_devices=NCORES)

    xgh_d = nc.dram_tensor("xgh", [P, CC, S], F8, kind="ExternalInput")
    xgl_d = nc.dram_tensor("xgl", [P, CC, S], F8, kind="ExternalInput")
    xsh_d = nc.dram_tensor("xsh", [P, CC, TS], F8, kind="ExternalInput")
    xsl_d = nc.dram_tensor("xsl", [P, CC, TS], F8, kind="ExternalInput")
    wuh_d = nc.dram_tensor("wuh", [P, CC, H], F8, kind="ExternalInput")
    wul_d = nc.dram_tensor("wul", [P, CC, H], F8, kind="ExternalInput")
    wgh_d = nc.dram_tensor("wgh", [P, CC, H], F8, kind="ExternalInput")
    wgl_d = nc.dram_tensor("wgl", [P, CC, H], F8, kind="ExternalInput")
    wdh_d = nc.dram_tensor("wdh", [P, HC, C], F8, kind="ExternalInput")
    wdl_d = nc.dram_tensor("wdl", [P, HC, C], F8, kind="ExternalInput")
    suh_d = nc.dram_tensor("suh", [P, CC, H], F8, kind="ExternalInput")
    sul_d = nc.dram_tensor("sul", [P, CC, H], F8, kind="ExternalInput")
    sgh_d = nc.dram_tensor("sgh", [P, CC, H], F8, kind="ExternalInput")
    sgl_d = nc.dram_tensor("sgl", [P, CC, H], F8, kind="ExternalInput")
    sdh_d = nc.dram_tensor("sdh", [P, HC, C], F8, kind="ExternalInput")
    sdl_d = nc.dram_tensor("sdl", [P, HC, C], F8, kind="ExternalInput")
    wslot_d = nc.dram_tensor("wslot", [P, SCH], F32, kind="ExternalInput")
    idx1_d = nc.dram_tensor("idx1", [P, TS // 16], I16, kind="ExternalInput")
    idx2_d = nc.dram_tensor("idx2", [P, TS // 16], I16, kind="ExternalInput")

    out_d = nc.dram_tensor("out", [TS, C], F32, kind="ExternalOutput")

    a2a_in = nc.dram_tensor("a2a_in", [S, C], BF16)
    a2a_out = nc.dram_tensor("a2a_out", [S, C], BF16)

    with tile.TileContext(nc) as tc:
        with (
            tc.tile_pool(name="pers", bufs=1) as pers,
            tc.tile_pool(name="psm", bufs=1, space="PSUM") as psm,
        ):
            xgh = pers.tile([P, CC, S], F8, name="xgh")
            xgl = pers.tile([P, CC, S], F8, name="xgl")
            xsh = pers.tile([P, CC, TS], F8, name="xsh")
            xsl = pers.tile([P, CC, TS], F8, name="xsl")
            wuh = pers.tile([P, CC, H], F8, name="wuh")
            wul = pers.tile([P, CC, H], F8, name="wul")
            wgh = pers.tile([P, CC, H], F8, name="wgh")
            wgl = pers.tile([P, CC, H], F8, name="wgl")
            wdh = pers.tile([P, HC, C], F8, name="wdh")
            wdl = pers.tile([P, HC, C], F8, name="wdl")
            ah = pers.tile([P, HC, S], F8, name="ah")
            al = pers.tile([P, HC, S], F8, name="al")
            wslot = pers.tile([P, SCH], F32, name="wslot")
            idx1 = pers.tile([P, TS // 16], I16, name="idx1")
            idx2 = pers.tile([P, TS // 16], I16, name="idx2")
            y1 = pers.tile([P, TS // P, C], BF16, name="y1")
            y2 = pers.tile([P, TS // P, C], BF16, name="y2")
            probe = pers.tile([16, 128], BF16, name="probe")

            def ps_up():
                return psm.tile([P, 512], F32, tag="up", bufs=2, name="up")

            def ps_gt():
                return psm.tile([P, 512], F32, tag="gt", bufs=2, name="gt")

            def ps_y():
                return psm.tile([P, 512], F32, tag="y", bufs=2, name="y")

            with tc.tile_pool(name="stg", bufs=1) as stg:
                def st_silu():
                    return stg.tile([P, 512], F32, tag="silu", bufs=2,
                                    name="silu")

                def st_prod():
                    return stg.tile([P, 512], F32, tag="prod", bufs=2,
                                    name="prod")

                def st_ysb():
                    return stg.tile([P, C], BF16, tag="ysb", bufs=2,
                                    name="ysb")

                def st_osb():
                    return stg.tile([P, 512], F32, tag="osb", bufs=6,
                                    name="osb")

                def load_w4(dsts, srcs, cl):
                    for d, s in zip(dsts, srcs):
                        nc.sync.dma_start(d[:, :, cl], s[:, :, cl])

                WUPG = (wuh, wul, wgh, wgl)

                # ---- initial loads: one strict need-ordered stream on
                # HWDGE; transfers are served in request order so the order
                # here IS the schedule. xs/qm/shared weights are deferred
                # past the first collective. ----
                RW = (wuh_d, wul_d, wgh_d, wgl_d)
                # first up-matmul operands first: xgh, wuh/wul, then xgl and
                # the gate weights, so the PE starts ~6us earlier
                nc.sync.dma_start(xgh[:, :, 0:512], xgh_d[:, :, 0:512])
                nc.sync.dma_start(wuh[:, :, 0:128], wuh_d[:, :, 0:128])
                nc.sync.dma_start(wul[:, :, 0:128], wul_d[:, :, 0:128])
                nc.sync.dma_start(xgl[:, :, 0:512], xgl_d[:, :, 0:512])
                nc.sync.dma_start(wgh[:, :, 0:128], wgh_d[:, :, 0:128])
                nc.sync.dma_start(wgl[:, :, 0:128], wgl_d[:, :, 0:128])
                load_w4(WUPG, RW, slice(128, 640))
                load_w4(WUPG, RW, slice(640, 1152))
                for x_t, x_d in ((xgh, xgh_d), (xgl, xgl_d)):
                    nc.sync.dma_start(x_t[:, :, 512:1024],
                                      x_d[:, :, 512:1024])
                load_w4(WUPG, RW, slice(1152, 1664))
                nc.sync.dma_start(wdh[:, :, 0:512], wdh_d[:, :, 0:512])
                nc.sync.dma_start(wdl[:, :, 0:512], wdl_d[:, :, 0:512])
                for x_t, x_d in ((xgh, xgh_d), (xgl, xgl_d)):
                    nc.sync.dma_start(x_t[:, :, 1024:S], x_d[:, :, 1024:S])
                load_w4(WUPG, RW, slice(1664, 2048))
                nc.sync.dma_start(wdh[:, :, 512:1024], wdh_d[:, :, 512:1024])
                nc.sync.dma_start(wdl[:, :, 512:1024], wdl_d[:, :, 512:1024])
                nc.sync.dma_start(wslot[:], wslot_d[:])
                nc.sync.dma_start(idx1[:], idx1_d[:])
                nc.sync.dma_start(idx2[:], idx2_d[:])

                def upgate(x_h, x_l, a, b, a_hi, a_lo):
                    """up/gate + silu*gate + hi/lo split for slots [a, b)."""
                    n = b - a
                    for hc in range(HC):
                        hsl = slice(hc * P, hc * P + P)
                        up_ps = ps_up()
                        gt_ps = ps_gt()
                        for ps, w_h, w_l in ((up_ps, wuh, wul),
                                             (gt_ps, wgh, wgl)):
                            terms = [(w_h, x_h), (w_h, x_l), (w_l, x_h)]
                            nmm = len(terms) * CPAIR
                            i = 0
                            for w_t, x_t in terms:
                                for j in range(CPAIR):
                                    nc.tensor.matmul(
                                        ps[:, 0:n],
                                        w_t[:, 2 * j:2 * j + 2, hsl],
                                        x_t[:, 2 * j:2 * j + 2, a:b],
                                        start=(i == 0), stop=(i == nmm - 1),
                                        perf_mode=DR)
                                    i += 1
                        silu = st_silu()
                        nc.scalar.activation(silu[:, 0:n], up_ps[:, 0:n],
                                             ACTF.Silu, scale=1.0 / SU)
                        prod = st_prod()
                        nc.vector.tensor_tensor(prod[:, 0:n], silu[:, 0:n],
                                                gt_ps[:, 0:n], op=OP.mult)
                        nc.scalar.activation(a_hi[:, hc, a:b], prod[:, 0:n],
                                             ACTF.Copy)
                        nc.vector.tensor_tensor(a_lo[:, hc, a:b],
                                                prod[:, 0:n],
                                                a_hi[:, hc, a:b],
                                                op=OP.subtract)

                def down_chunk(sc):
                    """split down matmul for routed slot chunk sc; stores the
                    wslot-scaled bf16 y rows to a2a_in."""
                    nr = min(P, S - sc * P)
                    ssl = slice(sc * P, sc * P + nr)
                    y_sb = st_ysb()
                    for cb in range(2):
                        csl = slice(cb * 512, (cb + 1) * 512)
                        y_ps = ps_y()
                        terms = [(ah, wdh), (ah, wdl), (al, wdh)]
                        nmm = len(terms) * HPAIR
                        i = 0
                        for a_t, w_t in terms:
                            for j in range(HPAIR):
                                nc.tensor.matmul(
                                    y_ps[0:nr, :],
                                    a_t[:, 2 * j:2 * j + 2, ssl],
                                    w_t[:, 2 * j:2 * j + 2, csl],
                                    start=(i == 0), stop=(i == nmm - 1),
                                    perf_mode=DR)
                                i += 1
                        nc.scalar.activation(y_sb[0:nr, csl], y_ps[0:nr, :],
                                             ACTF.Copy,
                                             scale=wslot[0:nr, sc:sc + 1])
                    nc.gpsimd.dma_start(a2a_in[sc * P:sc * P + nr, :],
                                        y_sb[0:nr, :])


                # shared up/gate weight chunks, streamed into the routed
                # tiles (WAR) during the last two phases
                sw_chunks = [(m, d, slice(hb * 512, (hb + 1) * 512))
                             for hb in range(4)
                             for m, d in ((wuh, suh_d), (wul, sul_d),
                                          (wgh, sgh_d), (wgl, sgl_d))]
                ci = 0

                # ---- routed expert: 3 pipelined phases ----
                nphase = len(PHASE_SC)
                for ph in range(nphase):
                    upgate(xgh, xgl, PHASE_UG[ph][0], PHASE_UG[ph][1],
                           ah, al)
                    for sc in range(PHASE_SC[ph][0], PHASE_SC[ph][1]):
                        down_chunk(sc)
                        if ph == nphase - 1:
                            # all routed upgate reads of wu/wg are done;
                            # stream the shared up/gate weights in
                            for _ in range(6):
                                if ci < len(sw_chunks):
                                    m, d, cl = sw_chunks[ci]
                                    nc.sync.dma_start(m[:, :, cl],
                                                      d[:, :, cl])
                                    ci += 1
                    nc.gpsimd.collective_compute(
                        "AllToAll", OP.bypass,
                        replica_groups=[list(range(NCORES))],
                        ins=[a2a_in[RSTART[ph]:RSTART[ph + 1], :]],
                        outs=[a2a_out[RSTART[ph]:RSTART[ph + 1], :]])
                    if ph == 0:
                        # deferred shared-expert inputs (needed from ~145us)
                        nc.scalar.dma_start(xsh[:], xsh_d[:])
                        nc.scalar.dma_start(xsl[:], xsl_d[:])
                while ci < len(sw_chunks):
                    m, d, cl = sw_chunks[ci]
                    nc.sync.dma_start(m[:, :, cl], d[:, :, cl])
                    ci += 1

                # tracked probe reads of both a2a extremes order the gathers
                # behind the collectives; then gather each token's two expert
                # rows (already wslot-weighted)
                nc.gpsimd.dma_start(probe[:, 64:128],
                                    a2a_out[S - 16:S, 0:64])
                nc.gpsimd.dma_gather(y1[:], a2a_out[:], idx1[:], TS, TS,
                                     elem_size=C)
                nc.gpsimd.dma_gather(y2[:], a2a_out[:], idx2[:], TS, TS,
                                     elem_size=C)

                # ---- shared expert up/gate (acts reuse ah/al[:, :, 0:TS]) --
                upgate(xsh, xsl, 0, TS, ah, al)
                # shared down weights into the routed down tiles
                nc.sync.dma_start(wdh[:], sdh_d[:])
                nc.sync.dma_start(wdl[:], sdl_d[:])

                # ---- shared down + combine ----
                for ts in range(TS // P):
                    tsl = slice(ts * P, (ts + 1) * P)
                    for cb in range(2):
                        csl = slice(cb * 512, (cb + 1) * 512)
                        y_ps = ps_y()
                        terms = [(ah, wdh), (ah, wdl), (al, wdh)]
                        nmm = len(terms) * HPAIR
                        i = 0
                        for a_t, w_t in terms:
                            for j in range(HPAIR):
                                nc.tensor.matmul(
                                    y_ps[:],
                                    a_t[:, 2 * j:2 * j + 2, tsl],
                                    w_t[:, 2 * j:2 * j + 2, csl],
                                    start=(i == 0), stop=(i == nmm - 1),
                                    perf_mode=DR)
                                i += 1
                        o_sb = st_osb()
                        nc.scalar.activation(o_sb[:], y_ps[:], ACTF.Copy,
                                             scale=1.0 / SY)
                        nc.vector.tensor_tensor(o_sb[:], o_sb[:],
                                                y1[:, ts, csl], op=OP.add)
                        eng2 = nc.gpsimd if ts * 2 + cb < 3 else nc.vector
                        eng2.tensor_tensor(o_sb[:], o_sb[:],
                                           y2[:, ts, csl], op=OP.add)
                        nc.sync.dma_start(out_d[tsl, csl], o_sb[:])

    nc.compile()
    return nc


_NC_CACHE = None


def _q8(a):
    a = np.clip(a, -240.0, 240.0)
    return a.astype(E4NP)


def _split8(a):
    hi = _q8(a)
    lo = _q8(a - hi.astype(np.float32))
    return hi, lo


def _idx_wrap(rows):
    """[TS] row indices -> [P, TS//16] int16 in the SWDGE gather layout
    (idx i at [i % 16, i // 16], replicated across the 16-row groups)."""
    m = np.asarray(rows, np.int16).reshape(TS // 16, 16).T
    return np.ascontiguousarray(np.tile(m, (P // 16, 1)))


def _dev3(a, nblk):
    """[K, F] -> [P, nblk, F] with global k = blk*128 + p."""
    k, f = a.shape
    assert k == nblk * P
    return np.ascontiguousarray(a.reshape(nblk, P, f).transpose(1, 0, 2))


def _idx_wrap(rows):
    """[TS] row indices -> [P, TS//16] int16 in the SWDGE gather layout
    (idx i at [i % 16, i // 16], replicated across the 16-row groups)."""
    m = np.asarray(rows, np.int16).reshape(TS // 16, 16).T  # [16, TS//16]
    return np.ascontiguousarray(np.tile(m, (P // 16, 1)))


def kernel(x, shared_Wup, shared_Wgate, shared_Wdown,
           routed_Wup, routed_Wgate, routed_Wdown, router_W):
    global _NC_CACHE
    if _NC_CACHE is None:
        _NC_CACHE = _build_program()
    nc = _NC_CACHE

    xf = np.ascontiguousarray(np.asarray(x, np.float32).reshape(T, C))
    rtw_m = np.ascontiguousarray(np.asarray(router_W, np.float32))

    logits = xf @ rtw_m
    top1 = np.argmax(logits, axis=1)
    l2 = logits.copy()
    l2[np.arange(T), top1] = -np.inf
    top2 = np.argmax(l2, axis=1)
    mx = logits.max(axis=1, keepdims=True)
    pr = np.exp(logits - mx)
    pr /= pr.sum(axis=1, keepdims=True)
    w1 = pr[np.arange(T), top1]
    w2 = pr[np.arange(T), top2]
    den = w1 + w2 + 1e-8
    w1, w2 = w1 / den, w2 / den

    # balanced ownership (greedy, deterministic)
    cnt = np.zeros((NCORES, E), np.int32)
    cap = np.full(NCORES, TS, np.int32)
    owner = np.empty(T, np.int32)
    order = np.argsort(top1 * E + top2, kind="stable")
    for t in order:
        a, b = top1[t], top2[t]
        best, bo = None, -1
        for o in range(NCORES):
            if cap[o] == 0:
                continue
            key = (max(cnt[o, a] + 1, cnt[o, b] + 1), cnt[o, a] + cnt[o, b],
                   -cap[o])
            if best is None or key < best:
                best, bo = key, o
        owner[t] = bo
        cnt[bo, a] += 1
        cnt[bo, b] += 1
        cap[bo] -= 1
    assert cnt.max() <= LC, f"balance failed: {cnt.max()} > {LC}"
    own_tokens = [np.sort(np.where(owner == o)[0]) for o in range(NCORES)]
    tok_pos = np.empty(T, np.int32)
    for o in range(NCORES):
        tok_pos[own_tokens[o]] = np.arange(TS)

    lists = [[[] for _ in range(NCORES)] for _ in range(E)]
    for t in range(T):
        o = owner[t]
        lists[top1[t]][o].append(t)
        lists[top2[t]][o].append(t)

    def slot_of(o, j):
        acc = 0
        for k, jl in enumerate(JSPLIT):
            if j < acc + jl:
                return RSTART[k] + o * jl + (j - acc)
            acc += jl
        raise AssertionError(j)

    def prep_w(w, scale, nblk):
        hi, lo = _split8(np.asarray(w, np.float32) * scale)
        return _dev3(hi, nblk), _dev3(lo, nblk)

    suh, sul = prep_w(shared_Wup, SU, CC)
    sgh, sgl = prep_w(shared_Wgate, SG, CC)
    sdh, sdl = prep_w(shared_Wdown, SD, HC)

    in_maps = []
    for c in range(NCORES):
        xg = np.zeros((S, C), np.float32)
        wvec = np.zeros(S, np.float32)
        for o in range(NCORES):
            for j, t in enumerate(lists[c][o]):
                sl = slot_of(o, j)
                xg[sl] = xf[t]
                wvec[sl] = w1[t] if top1[t] == c else w2[t]
        row1 = np.zeros(TS, np.int64)
        row2 = np.zeros(TS, np.int64)
        for e in range(E):
            for j, t in enumerate(lists[e][c]):
                r = slot_of(e, j)
                if top1[t] == e:
                    row1[tok_pos[t]] = r
                else:
                    row2[tok_pos[t]] = r
        xgh, xgl = _split8(xg.T)
        xs = xf[own_tokens[c], :].T
        xsh, xsl = _split8(xs)
        ruh, rul = prep_w(routed_Wup[c], SU, CC)
        rgh, rgl = prep_w(routed_Wgate[c], SG, CC)
        rdh, rdl = prep_w(routed_Wdown[c], SD, HC)
        wpad = np.zeros(SP_, np.float32)
        wpad[:S] = wvec / SY
        wslot_m = np.ascontiguousarray(
            wpad.reshape(SCH, P).T.astype(np.float32))
        in_maps.append({
            "xgh": _dev3(xgh, CC), "xgl": _dev3(xgl, CC),
            "xsh": _dev3(xsh, CC), "xsl": _dev3(xsl, CC),
            "wuh": ruh, "wul": rul, "wgh": rgh, "wgl": rgl,
            "wdh": rdh, "wdl": rdl,
            "suh": suh, "sul": sul, "sgh": sgh, "sgl": sgl,
            "sdh": sdh, "sdl": sdl,
            "wslot": wslot_m,
            "idx1": _idx_wrap(row1), "idx2": _idx_wrap(row2),
        })

    res = run_bass_kernel_spmd(nc, in_maps, list(range(NCORES)))
    full = np.empty((T, C), np.float32)
    for c in range(NCORES):
        full[own_tokens[c]] = res.results[c]["out"]
    return full.reshape(2, 2048, C).astype(np.float32)


# revision 3
# speedup vs baseline: 1.0371x; 1.0371x over previous
"""MoE FFN (8 routed experts top-2 + 1 shared) on 8 TRN2 NeuronCores.

Expert-parallel token dispatch with split-fp8 DoubleRow matmuls (hi+lo
e4m3 pairs, 3 DR matmuls per 256-contraction: 4/3 over bf16 at bf16
accuracy). Router/combine weights on host. v3 over v2:
 - a2a halves rebalanced (JH=88: 704/448 rows) so the second, critical
   collective is smaller and starts as early as possible
 - shared-expert weight streams are chunked and interleaved with the
   second down phase so a2a_in stores don't queue behind them
 - startup loads trimmed (first up/gate weight chunk is 128 cols)
 - combine via dma_gather of the two expert rows per token + DVE adds
   (removes the one-hot combine matmul from the PE critical path)
"""

import ml_dtypes
import numpy as np

import concourse.bacc as bacc
import concourse.bass as bass
import concourse.mybir as mybir
import concourse.tile as tile
from concourse.bass_utils import run_bass_kernel_spmd

P = 128
C = 1024            # d_model
H = 2048            # d_expert
T = 4096            # tokens
E = 8               # routed experts == cores
TS = T // E         # 512 tokens per core
CC = C // P         # 8 k-blocks for up/gate contraction
CPAIR = CC // 2     # 4 DoubleRow pairs
HC = H // P         # 16 k-blocks for down contraction
HPAIR = HC // 2     # 8 DoubleRow pairs
LC = 138            # per (expert, owner) capacity (host balances ownership)
S = E * LC          # 1104 slots per expert core
SCH = (S + P - 1) // P      # 9 slot chunks (last partial: 80 slots)
SP_ = SCH * P       # 1152: padded slot count for [P, SCH] host layouts
JSPLIT = (48, 48, 42)       # per-owner slots per a2a chunk (sum = LC)
RSTART = (0, 384, 768, 1104)    # chunk row boundaries (E * cumsum)
PHASE_SC = ((0, 3), (3, 6), (6, 9))   # down slot-chunks per phase
PHASE_UG = ((0, 384), (384, 768), (768, 1104))

NCORES = 8
F32 = mybir.dt.float32
BF16 = mybir.dt.bfloat16
I16 = mybir.dt.int16
F8 = mybir.dt.float8e4
DR = mybir.MatmulPerfMode.DoubleRow
ACTF = mybir.ActivationFunctionType
OP = mybir.AluOpType
E4NP = ml_dtypes.float8_e4m3

SU = 64.0           # Wup scale
SG = 8.0            # Wgate scale (act tiles carry this scale)
SD = 64.0           # Wdown scale
SY = SG * SD        # 512: PSUM y scale


def _build_program():
    nc = bacc.Bacc("TRN2", target_bir_lowering=False, debug=False,
                   num_devices=NCORES)

    xgh_d = nc.dram_tensor("xgh", [P, CC, S], F8, kind="ExternalInput")
    xgl_d = nc.dram_tensor("xgl", [P, CC, S], F8, kind="ExternalInput")
    xsh_d = nc.dram_tensor("xsh", [P, CC, TS], F8, kind="ExternalInput")
    xsl_d = nc.dram_tensor("xsl", [P, CC, TS], F8, kind="ExternalInput")
    wuh_d = nc.dram_tensor("wuh", [P, CC, H], F8, kind="ExternalInput")
    wul_d = nc.dram_tensor("wul", [P, CC, H], F8, kind="ExternalInput")
    wgh_d = nc.dram_tensor("wgh", [P, CC, H], F8, kind="ExternalInput")
    wgl_d = nc.dram_tensor("wgl", [P, CC, H], F8, kind="ExternalInput")
    wdh_d = nc.dram_tensor("wdh", [P, HC, C], F8, kind="ExternalInput")
    wdl_d = nc.dram_tensor("wdl", [P, HC, C], F8, kind="ExternalInput")
    suh_d = nc.dram_tensor("suh", [P, CC, H], F8, kind="ExternalInput")
    sul_d = nc.dram_tensor("sul", [P, CC, H], F8, kind="ExternalInput")
    sgh_d = nc.dram_tensor("sgh", [P, CC, H], F8, kind="ExternalInput")
    sgl_d = nc.dram_tensor("sgl", [P, CC, H], F8, kind="ExternalInput")
    sdh_d = nc.dram_tensor("sdh", [P, HC, C], F8, kind="ExternalInput")
    sdl_d = nc.dram_tensor("sdl", [P, HC, C], F8, kind="ExternalInput")
    wslot_d = nc.dram_tensor("wslot", [P, SCH], F32, kind="ExternalInput")
    idx1_d = nc.dram_tensor("idx1", [P, TS // 16], I16, kind="ExternalInput")
    idx2_d = nc.dram_tensor("idx2", [P, TS // 16], I16, kind="ExternalInput")

    out_d = nc.dram_tensor("out", [TS, C], F32, kind="ExternalOutput")

    a2a_in = nc.dram_tensor("a2a_in", [S, C], BF16)
    a2a_out = nc.dram_tensor("a2a_out", [S, C], BF16)

    with tile.TileContext(nc) as tc:
        with (
            tc.tile_pool(name="pers", bufs=1) as pers,
            tc.tile_pool(name="psm", bufs=1, space="PSUM") as psm,
        ):
            xgh = pers.tile([P, CC, S], F8, name="xgh")
            xgl = pers.tile([P, CC, S], F8, name="xgl")
            xsh = pers.tile([P, CC, TS], F8, name="xsh")
            xsl = pers.tile([P, CC, TS], F8, name="xsl")
            wuh = pers.tile([P, CC, H], F8, name="wuh")
            wul = pers.tile([P, CC, H], F8, name="wul")
            wgh = pers.tile([P, CC, H], F8, name="wgh")
            wgl = pers.tile([P, CC, H], F8, name="wgl")
            wdh = pers.tile([P, HC, C], F8, name="wdh")
            wdl = pers.tile([P, HC, C], F8, name="wdl")
            ah = pers.tile([P, HC, S], F8, name="ah")
            al = pers.tile([P, HC, S], F8, name="al")
            wslot = pers.tile([P, SCH], F32, name="wslot")
            idx1 = pers.tile([P, TS // 16], I16, name="idx1")
            idx2 = pers.tile([P, TS // 16], I16, name="idx2")
            y1 = pers.tile([P, TS // P, C], BF16, name="y1")
            y2 = pers.tile([P, TS // P, C], BF16, name="y2")
            probe = pers.tile([16, 128], BF16, name="probe")

            def ps_up():
                return psm.tile([P, 512], F32, tag="up", bufs=2, name="up")

            def ps_gt():
                return psm.tile([P, 512], F32, tag="gt", bufs=2, name="gt")

            def ps_y():
                return psm.tile([P, 512], F32, tag="y", bufs=2, name="y")

            with tc.tile_pool(name="stg", bufs=1) as stg:
                def st_silu():
                    return stg.tile([P, 512], F32, tag="silu", bufs=2,
                                    name="silu")

                def st_prod():
                    return stg.tile([P, 512], F32, tag="prod", bufs=2,
                                    name="prod")

                def st_ysb():
                    return stg.tile([P, C], BF16, tag="ysb", bufs=2,
                                    name="ysb")

                def st_osb():
                    return stg.tile([P, 512], F32, tag="osb", bufs=6,
                                    name="osb")

                def load_w4(dsts, srcs, cl):
                    for d, s in zip(dsts, srcs):
                        nc.sync.dma_start(d[:, :, cl], s[:, :, cl])

                WUPG = (wuh, wul, wgh, wgl)

                # ---- initial loads: one strict need-ordered stream on
                # HWDGE; transfers are served in request order so the order
                # here IS the schedule. xs/qm/shared weights are deferred
                # past the first collective. ----
                RW = (wuh_d, wul_d, wgh_d, wgl_d)
                # first up-matmul operands first: xgh, wuh/wul, then xgl and
                # the gate weights, so the PE starts ~6us earlier
                nc.sync.dma_start(xgh[:, :, 0:512], xgh_d[:, :, 0:512])
                nc.sync.dma_start(wuh[:, :, 0:128], wuh_d[:, :, 0:128])
                nc.sync.dma_start(wul[:, :, 0:128], wul_d[:, :, 0:128])
                nc.sync.dma_start(xgl[:, :, 0:512], xgl_d[:, :, 0:512])
                nc.sync.dma_start(wgh[:, :, 0:128], wgh_d[:, :, 0:128])
                nc.sync.dma_start(wgl[:, :, 0:128], wgl_d[:, :, 0:128])
                load_w4(WUPG, RW, slice(128, 640))
                load_w4(WUPG, RW, slice(640, 1152))
                for x_t, x_d in ((xgh, xgh_d), (xgl, xgl_d)):
                    nc.sync.dma_start(x_t[:, :, 512:1024],
                                      x_d[:, :, 512:1024])
                load_w4(WUPG, RW, slice(1152, 1664))
                nc.sync.dma_start(wdh[:, :, 0:512], wdh_d[:, :, 0:512])
                nc.sync.dma_start(wdl[:, :, 0:512], wdl_d[:, :, 0:512])
                for x_t, x_d in ((xgh, xgh_d), (xgl, xgl_d)):
                    nc.sync.dma_start(x_t[:, :, 1024:S], x_d[:, :, 1024:S])
                load_w4(WUPG, RW, slice(1664, 2048))
                nc.sync.dma_start(wdh[:, :, 512:1024], wdh_d[:, :, 512:1024])
                nc.sync.dma_start(wdl[:, :, 512:1024], wdl_d[:, :, 512:1024])
                nc.sync.dma_start(wslot[:], wslot_d[:])
                nc.sync.dma_start(idx1[:], idx1_d[:])
                nc.sync.dma_start(idx2[:], idx2_d[:])

                def upgate(x_h, x_l, a, b, a_hi, a_lo):
                    """up/gate + silu*gate + hi/lo split for slots [a, b)."""
                    n = b - a
                    for hc in range(HC):
                        hsl = slice(hc * P, hc * P + P)
                        up_ps = ps_up()
                        gt_ps = ps_gt()
                        for ps, w_h, w_l in ((up_ps, wuh, wul),
                                             (gt_ps, wgh, wgl)):
                            terms = [(w_h, x_h), (w_h, x_l), (w_l, x_h)]
                            nmm = len(terms) * CPAIR
                            i = 0
                            for w_t, x_t in terms:
                                for j in range(CPAIR):
                                    nc.tensor.matmul(
                                        ps[:, 0:n],
                                        w_t[:, 2 * j:2 * j + 2, hsl],
                                        x_t[:, 2 * j:2 * j + 2, a:b],
                                        start=(i == 0), stop=(i == nmm - 1),
                                        perf_mode=DR)
                                    i += 1
                        silu = st_silu()
                        nc.scalar.activation(silu[:, 0:n], up_ps[:, 0:n],
                                             ACTF.Silu, scale=1.0 / SU)
                        prod = st_prod()
                        nc.vector.tensor_tensor(prod[:, 0:n], silu[:, 0:n],
                                                gt_ps[:, 0:n], op=OP.mult)
                        nc.scalar.activation(a_hi[:, hc, a:b], prod[:, 0:n],
                                             ACTF.Copy)
                        nc.vector.tensor_tensor(a_lo[:, hc, a:b],
                                                prod[:, 0:n],
                                                a_hi[:, hc, a:b],
                                                op=OP.subtract)

                def down_chunk(sc):
                    """split down matmul for routed slot chunk sc; stores the
                    wslot-scaled bf16 y rows to a2a_in."""
                    nr = min(P, S - sc * P)
                    ssl = slice(sc * P, sc * P + nr)
                    y_sb = st_ysb()
                    for cb in range(2):
                        csl = slice(cb * 512, (cb + 1) * 512)
                        y_ps = ps_y()
                        terms = [(ah, wdh), (ah, wdl), (al, wdh)]
                        nmm = len(terms) * HPAIR
                        i = 0
                        for a_t, w_t in terms:
                            for j in range(HPAIR):
                                nc.tensor.matmul(
                                    y_ps[0:nr, :],
                                    a_t[:, 2 * j:2 * j + 2, ssl],
                                    w_t[:, 2 * j:2 * j + 2, csl],
                                    start=(i == 0), stop=(i == nmm - 1),
                                    perf_mode=DR)
                                i += 1
                        nc.scalar.activation(y_sb[0:nr, csl], y_ps[0:nr, :],
                                             ACTF.Copy,
                                             scale=wslot[0:nr, sc:sc + 1])
                    nc.gpsimd.dma_start(a2a_in[sc * P:sc * P + nr, :],
                                        y_sb[0:nr, :])


                # shared up/gate weight chunks, streamed into the routed
                # tiles (WAR) during the last two phases
                sw_chunks = [(m, d, slice(hb * 512, (hb + 1) * 512))
                             for hb in range(4)
                             for m, d in ((wuh, suh_d), (wul, sul_d),
                                          (wgh, sgh_d), (wgl, sgl_d))]
                ci = 0

                # ---- routed expert: 3 pipelined phases ----
                nphase = len(PHASE_SC)
                for ph in range(nphase):
                    upgate(xgh, xgl, PHASE_UG[ph][0], PHASE_UG[ph][1],
                           ah, al)
                    for sc in range(PHASE_SC[ph][0], PHASE_SC[ph][1]):
                        down_chunk(sc)
                        if ph == nphase - 1:
                            # all routed upgate reads of wu/wg are done;
                            # stream the shared up/gate weights in
                            for _ in range(6):
                                if ci < len(sw_chunks):
                                    m, d, cl = sw_chunks[ci]
                                    nc.sync.dma_start(m[:, :, cl],
                                                      d[:, :, cl])
                                    ci += 1
                    nc.gpsimd.collective_compute(
                        "AllToAll", OP.bypass,
                        replica_groups=[list(range(NCORES))],
                        ins=[a2a_in[RSTART[ph]:RSTART[ph + 1], :]],
                        outs=[a2a_out[RSTART[ph]:RSTART[ph + 1], :]])
                    if ph == 0:
                        # deferred shared-expert inputs (needed from ~145us)
                        nc.scalar.dma_start(xsh[:], xsh_d[:])
                        nc.scalar.dma_start(xsl[:], xsl_d[:])
                while ci < len(sw_chunks):
                    m, d, cl = sw_chunks[ci]
                    nc.sync.dma_start(m[:, :, cl], d[:, :, cl])
                    ci += 1

                # tracked probe reads of both a2a extremes order the gathers
                # behind the collectives; then gather each token's two expert
                # rows (already wslot-weighted)
                nc.gpsimd.dma_start(probe[:, 64:128],
                                    a2a_out[S - 16:S, 0:64])
                nc.gpsimd.dma_gather(y1[:], a2a_out[:], idx1[:], TS, TS,
                                     elem_size=C)
                nc.gpsimd.dma_gather(y2[:], a2a_out[:], idx2[:], TS, TS,
                                     elem_size=C)

                # ---- shared expert up/gate (acts reuse ah/al[:, :, 0:TS]) --
                upgate(xsh, xsl, 0, TS, ah, al)
                # shared down weights into the routed down tiles
                nc.sync.dma_start(wdh[:], sdh_d[:])
                nc.sync.dma_start(wdl[:], sdl_d[:])

                # ---- shared down + combine (last group split small so the
                # final epilogue chain drains quickly) ----
                cparts = [(ts, slice(cb * 512, (cb + 1) * 512), ts * 2 + cb)
                          for ts in range(TS // P) for cb in range(2)]
                cparts = cparts[:-1] + [(3, slice(512, 896), 7),
                                        (3, slice(896, 1024), 7)]
                for ts, csl, gi in cparts:
                    tsl = slice(ts * P, (ts + 1) * P)
                    if True:
                        nw = csl.stop - csl.start
                        y_ps = ps_y()
                        terms = [(ah, wdh), (ah, wdl), (al, wdh)]
                        nmm = len(terms) * HPAIR
                        i = 0
                        for a_t, w_t in terms:
                            for j in range(HPAIR):
                                nc.tensor.matmul(
                                    y_ps[:, 0:nw],
                                    a_t[:, 2 * j:2 * j + 2, tsl],
                                    w_t[:, 2 * j:2 * j + 2, csl],
                                    start=(i == 0), stop=(i == nmm - 1),
                                    perf_mode=DR)
                                i += 1
                        o_sb = st_osb()
                        nc.scalar.activation(o_sb[:, 0:nw], y_ps[:, 0:nw],
                                             ACTF.Copy, scale=1.0 / SY)
                        nc.vector.tensor_tensor(o_sb[:, 0:nw],
                                                o_sb[:, 0:nw],
                                                y1[:, ts, csl], op=OP.add)
                        eng2 = nc.gpsimd if gi < 3 else nc.vector
                        eng2.tensor_tensor(o_sb[:, 0:nw], o_sb[:, 0:nw],
                                           y2[:, ts, csl], op=OP.add)
                        nc.sync.dma_start(out_d[tsl, csl], o_sb[:, 0:nw])

    nc.compile()
    return nc


_NC_CACHE = None


def _q8(a):
    a = np.clip(a, -240.0, 240.0)
    return a.astype(E4NP)


def _split8(a):
    hi = _q8(a)
    lo = _q8(a - hi.astype(np.float32))
    return hi, lo


def _idx_wrap(rows):
    """[TS] row indices -> [P, TS//16] int16 in the SWDGE gather layout
    (idx i at [i % 16, i // 16], replicated across the 16-row groups)."""
    m = np.asarray(rows, np.int16).reshape(TS // 16, 16).T
    return np.ascontiguousarray(np.tile(m, (P // 16, 1)))


def _dev3(a, nblk):
    """[K, F] -> [P, nblk, F] with global k = blk*128 + p."""
    k, f = a.shape
    assert k == nblk * P
    return np.ascontiguousarray(a.reshape(nblk, P, f).transpose(1, 0, 2))


def _idx_wrap(rows):
    """[TS] row indices -> [P, TS//16] int16 in the SWDGE gather layout
    (idx i at [i % 16, i // 16], replicated across the 16-row groups)."""
    m = np.asarray(rows, np.int16).reshape(TS // 16, 16).T  # [16, TS//16]
    return np.ascontiguousarray(np.tile(m, (P // 16, 1)))


def kernel(x, shared_Wup, shared_Wgate, shared_Wdown,
           routed_Wup, routed_Wgate, routed_Wdown, router_W):
    global _NC_CACHE
    if _NC_CACHE is None:
        _NC_CACHE = _build_program()
    nc = _NC_CACHE

    xf = np.ascontiguousarray(np.asarray(x, np.float32).reshape(T, C))
    rtw_m = np.ascontiguousarray(np.asarray(router_W, np.float32))

    logits = xf @ rtw_m
    top1 = np.argmax(logits, axis=1)
    l2 = logits.copy()
    l2[np.arange(T), top1] = -np.inf
    top2 = np.argmax(l2, axis=1)
    mx = logits.max(axis=1, keepdims=True)
    pr = np.exp(logits - mx)
    pr /= pr.sum(axis=1, keepdims=True)
    w1 = pr[np.arange(T), top1]
    w2 = pr[np.arange(T), top2]
    den = w1 + w2 + 1e-8
    w1, w2 = w1 / den, w2 / den

    # balanced ownership (greedy, deterministic)
    cnt = np.zeros((NCORES, E), np.int32)
    cap = np.full(NCORES, TS, np.int32)
    owner = np.empty(T, np.int32)
    order = np.argsort(top1 * E + top2, kind="stable")
    for t in order:
        a, b = top1[t], top2[t]
        best, bo = None, -1
        for o in range(NCORES):
            if cap[o] == 0:
                continue
            key = (max(cnt[o, a] + 1, cnt[o, b] + 1), cnt[o, a] + cnt[o, b],
                   -cap[o])
            if best is None or key < best:
                best, bo = key, o
        owner[t] = bo
        cnt[bo, a] += 1
        cnt[bo, b] += 1
        cap[bo] -= 1
    assert cnt.max() <= LC, f"balance failed: {cnt.max()} > {LC}"
    own_tokens = [np.sort(np.where(owner == o)[0]) for o in range(NCORES)]
    tok_pos = np.empty(T, np.int32)
    for o in range(NCORES):
        tok_pos[own_tokens[o]] = np.arange(TS)

    lists = [[[] for _ in range(NCORES)] for _ in range(E)]
    for t in range(T):
        o = owner[t]
        lists[top1[t]][o].append(t)
        lists[top2[t]][o].append(t)

    def slot_of(o, j):
        acc = 0
        for k, jl in enumerate(JSPLIT):
            if j < acc + jl:
                return RSTART[k] + o * jl + (j - acc)
            acc += jl
        raise AssertionError(j)

    def prep_w(w, scale, nblk):
        hi, lo = _split8(np.asarray(w, np.float32) * scale)
        return _dev3(hi, nblk), _dev3(lo, nblk)

    suh, sul = prep_w(shared_Wup, SU, CC)
    sgh, sgl = prep_w(shared_Wgate, SG, CC)
    sdh, sdl = prep_w(shared_Wdown, SD, HC)

    in_maps = []
    for c in range(NCORES):
        xg = np.zeros((S, C), np.float32)
        wvec = np.zeros(S, np.float32)
        for o in range(NCORES):
            for j, t in enumerate(lists[c][o]):
                sl = slot_of(o, j)
                xg[sl] = xf[t]
                wvec[sl] = w1[t] if top1[t] == c else w2[t]
        row1 = np.zeros(TS, np.int64)
        row2 = np.zeros(TS, np.int64)
        for e in range(E):
            for j, t in enumerate(lists[e][c]):
                r = slot_of(e, j)
                if top1[t] == e:
                    row1[tok_pos[t]] = r
                else:
                    row2[tok_pos[t]] = r
        xgh, xgl = _split8(xg.T)
        xs = xf[own_tokens[c], :].T
        xsh, xsl = _split8(xs)
        ruh, rul = prep_w(routed_Wup[c], SU, CC)
        rgh, rgl = prep_w(routed_Wgate[c], SG, CC)
        rdh, rdl = prep_w(routed_Wdown[c], SD, HC)
        wpad = np.zeros(SP_, np.float32)
        wpad[:S] = wvec / SY
        wslot_m = np.ascontiguousarray(
            wpad.reshape(SCH, P).T.astype(np.float32))
        in_maps.append({
            "xgh": _dev3(xgh, CC), "xgl": _dev3(xgl, CC),
            "xsh": _dev3(xsh, CC), "xsl": _dev3(xsl, CC),
            "wuh": ruh, "wul": rul, "wgh": rgh, "wgl": rgl,
            "wdh": rdh, "wdl": rdl,
            "suh": suh, "sul": sul, "sgh": sgh, "sgl": sgl,
            "sdh": sdh, "sdl": sdl,
            "wslot": wslot_m,
            "idx1": _idx_wrap(row1), "idx2": _idx_wrap(row2),
        })

    res = run_bass_kernel_spmd(nc, in_maps, list(range(NCORES)))
    full = np.empty((T, C), np.float32)
    for c in range(NCORES):
        full[own_tokens[c]] = res.results[c]["out"]
    return full.reshape(2, 2048, C).astype(np.float32)
